# revision 49
# baseline (speedup 1.0000x reference)
"""Trainium2 Bass kernel for the AnaphoricityScorer problem.

Data-parallel over the batch (mention) dimension across 8 NeuronCores.
Per core: 64 mentions x 50 antecedents = 3200 pair rows, r = ant*64 + m.

pair = [a, b, a*b, pw] @ W1 with the a-term folded into a fused k-tile
(T_a' = mentions @ W1a + b1 injected through a 0/1 selection matrix S).
The b and a*b terms run as fp8(e4m3) DoubleRow matmuls (K=256 per
instruction at 1 output row/cycle, 2x bf16 throughput); to condition
the fp8 weights, the whole pre-activation is scaled by 64 (W1*64,
b1*64 exact power-of-2 scalings) and undone through W2/64 - valid
because LeakyReLU is positively homogeneous. Gathers, transposes and
the fused tile run in bf16. Gathered rows transpose on the
TensorEngine (bf16, 1 cycle/row) into PSUM; per 128-row tile one fp8
cast (alternating Scalar/DVE) makes the b^T slabs and one DVE multiply
makes the fp8 (a*b)^T slabs, laid out [128, 2, N] for DoubleRow
consumption.

Row chunks of [256, 256, 512x5, 128] accumulate full-width in one PSUM
bank per (hid-tile, chunk) unit so each DoubleRow LDWEIGHTS (256
weight cols ~ 256 cycles) feeds a full 512-row moving stream. The w2
reduction batches at chunk end with two entries deferred into the next
chunk (emit_out rides the flush), keeping the PE off the Lrelu
critical path and minimizing fp8<->bf16 weight-mode flips. Startup
streams are spread across the Sync (fp8 weights + pwS) and Scalar
(idx + W1a quarters) DGE queues while indirect gathers own the Pool
queue. USE_YB gates an experimental AllGather-based Y_b=am@W1b
precompute path (disabled: NEFF load fails; see session notes).
"""

import os
import sys
from contextlib import ExitStack

import numpy as np
import ml_dtypes

for _p in ("/opt/trn_rl_repo",):
    if _p not in sys.path and os.path.isdir(_p):
        sys.path.insert(0, _p)

from concourse import bass, mybir  # noqa: E402
import concourse.tile as tile  # noqa: E402
from concourse.masks import make_identity  # noqa: E402
from concourse.bass_utils import run_bass_kernel_spmd  # noqa: E402

NM, BATCH, A, E, PW, HID, NCORES = 2000, 512, 50, 1024, 64, 1024, 8
BS = BATCH // NCORES
R = A * BS
USE_YB = False  # Y_b AllGather inject path (crashes NEFF load; see memory)
YS = NM // NCORES  # Y_b precompute rows per core
YC0 = 3 if USE_YB else 10**9  # first chunk using the Y_b inject path
ALPHA, EPSILON = 0.01, 1e-07
SC = 64.0
F32 = mybir.dt.float32
BF16 = mybir.dt.bfloat16
FP8 = mybir.dt.float8e4
I32 = mybir.dt.int32
DRM = mybir.MatmulPerfMode.DoubleRow
KE, NT, NG = E // 128, HID // 128, E // 256

_CH = [256, 256, 512, 512, 512, 512, 512, 128]
_RCS = [0, 256, 512, 1024, 1536, 2048, 2560, 3072]
NCHUNK = len(_CH)

BF_NP = ml_dtypes.bfloat16
F8_NP = ml_dtypes.float8_e4m3


def _redistribute_waits(nc, helper_sems, limit=1):
    """Enforce <=1 sync wait per instruction (walrus limit on this build).

    Compute-engine instructions execute in-order on their engine stream, so
    excess waits hoist into single-wait InstEventSemaphore instructions
    spliced just before them. DMACopy instructions execute from concurrent
    DGE queue programs, so an engine-stream EventSem does NOT gate them:
    their waits are bridged through a per-engine helper semaphore - the
    EventSems consume the original waits on the engine stream and increment
    the helper; the DMA's single wait slot watches the helper's cumulative
    count. Helpers are decremented back to zero at the end so repeated
    executions of the loaded NEFF stay correct.
    """
    counter = [0]
    counts = {e: 0 for e in helper_sems}
    last_dma = {}
    last_blk = None

    def mk_ev(engine, wait=None, update=None):
        ev = mybir.InstEventSemaphore(
            name=f"hoistw-{counter[0]}", ins=[], outs=[]
        )
        counter[0] += 1
        ev.engine = engine
        ev.sync_info = mybir.SyncInfo(
            on_wait=[wait] if wait else [], on_update=[update] if update else []
        )
        return ev

    for f in nc.m.functions:
        for blk in f.blocks:
            il = blk.instructions
            if il:
                last_blk = blk
            new_il = []
            changed = False
            for inst in il:
                si = inst.sync_info
                waits = list(si.on_wait) if si is not None else []
                if isinstance(inst, mybir.InstDMACopy) and len(waits) > limit:
                    h = helper_sems[inst.engine]
                    for i, w in enumerate(waits):
                        upd = None
                        if i == len(waits) - 1:
                            upd = mybir.SyncUpdate(
                                sync_type="semaphore",
                                id=h.num,
                                ant_name=h.name,
                                update_mode="sem-inc",
                                update_value=1,
                            )
                        new_il.append(mk_ev(inst.engine, w, upd))
                    counts[inst.engine] += 1
                    last_dma[inst.engine] = inst
                    si.on_wait = [
                        mybir.SyncWait(
                            sync_type="semaphore",
                            id=h.num,
                            ant_name=h.name,
                            wait_mode="sem-ge-imm",
                            wait_value=counts[inst.engine],
                        )
                    ]
                    changed = True
                elif len(waits) > limit:
                    for w in waits[:-limit]:
                        new_il.append(mk_ev(inst.engine, w))
                    si.on_wait = waits[-limit:]
                    changed = True
                new_il.append(inst)
            if changed:
                blk.instructions = new_il

    # Reset each helper to zero right AFTER that engine's last hoisted DMA
    # (cumulative thresholds make earlier placement unsafe, and
    # end-of-stream placement would serialize the reset into the kernel
    # tail). A single write-immediate replaces the old N-deep decrement
    # chain, which serialized ~100ns/instruction into the tail barrier.
    def decs(eng, h, n):
        return [
            mk_ev(
                eng,
                None,
                mybir.SyncUpdate(
                    sync_type="semaphore",
                    id=h.num,
                    ant_name=h.name,
                    update_mode="sem-wr-imm",
                    update_value=0,
                ),
            )
        ]

    for f in nc.m.functions:
        for blk in f.blocks:
            il = blk.instructions
            new_il = []
            changed = False
            for inst in il:
                new_il.append(inst)
                for eng, h in helper_sems.items():
                    if counts[eng] and last_dma.get(eng) is inst:
                        new_il.extend(decs(eng, h, counts[eng]))
                        changed = True
            if changed:
                blk.instructions = new_il


def build_nc():
    nc = bass.Bass("TRN2", target_bir_lowering=False, debug=False,
                   num_devices=NCORES)
    am_d = nc.declare_dram_parameter("am", [NM, E], BF16, isOutput=False)
    amt_d = nc.declare_dram_parameter("amt", [128, KE * YS], BF16, isOutput=False)
    ys_d = nc.dram_tensor("ysl", [YS, HID], BF16, kind="Internal")
    yf_d = nc.dram_tensor("yfl", [NM, HID], BF16, kind="Internal")
    id_d = nc.declare_dram_parameter("idb", [128, 128], BF16, isOutput=False)
    mts_d = nc.declare_dram_parameter("mts", [128, KE * BS], BF16, isOutput=False)
    pwS_d = nc.declare_dram_parameter("pwS", [128, R], BF16, isOutput=False)
    idx_d = nc.declare_dram_parameter("idx", [128, R // 128], I32, isOutput=False)
    rough_d = nc.declare_dram_parameter("rough", [1, R], F32, isOutput=False)
    w1a_d = nc.declare_dram_parameter("w1a", [128, KE * HID], BF16, isOutput=False)
    w1b8_d = nc.declare_dram_parameter("w1b8", [128, NT * E], FP8, isOutput=False)
    w1c8_d = nc.declare_dram_parameter("w1c8", [128, NT * E], FP8, isOutput=False)
    w1d_d = nc.declare_dram_parameter("w1d", [PW, HID], BF16, isOutput=False)
    w2r_d = nc.declare_dram_parameter("w2r", [128, NT], BF16, isOutput=False)
    b1r_d = nc.declare_dram_parameter("b1r", [1, HID], BF16, isOutput=False)
    out_d = nc.declare_dram_parameter("out", [1, R], F32, isOutput=True)

    helper_sems = {
        mybir.EngineType.SP: nc.alloc_semaphore("hoist_dma_sp"),
        mybir.EngineType.Pool: nc.alloc_semaphore("hoist_dma_pool"),
        mybir.EngineType.Activation: nc.alloc_semaphore("hoist_dma_act"),
    }

    with tile.TileContext(nc) as tc:
        with ExitStack() as ctx:
            const = ctx.enter_context(tc.tile_pool(name="const", bufs=1))
            w1b8_sb = const.tile([128, NT * E], FP8, tag="w1b8")
            w1c8_sb = const.tile([128, NT * E], FP8, tag="w1c8")
            wfused = const.tile([128, HID], BF16, tag="wfused")
            pwS_sb = const.tile([128, R], BF16, tag="pwS")
            mts_sb = const.tile([128, KE * BS], BF16, tag="mts")
            w2_sb = const.tile([128, NT], BF16, tag="w2")
            b1_sb = const.tile([1, HID], BF16, tag="b1")
            identb = const.tile([128, 128], BF16, tag="identb")
            idx_sb = const.tile([128, R // 128], I32, tag="idx")
            rough_sb = const.tile([1, R], F32, tag="rough")
            ones_f = const.tile([1, BS], F32, tag="ones_f")
            ones_b = const.tile([1, BS], BF16, tag="ones_b")

            # small consts first on the SP DMA queue
            nc.sync.dma_start(identb[:], id_d[:])
            nc.sync.dma_start(b1_sb[:], b1r_d[:])
            nc.sync.dma_start(mts_sb[:], mts_d[:])
            nc.sync.dma_start(w2_sb[:], w2r_d[:])
            nc.sync.dma_start(wfused[0:PW, :], w1d_d[:])
            nc.sync.dma_start(rough_sb[:], rough_d[:])
            # idx is a [128, 25] strided write: descriptor-bound, so split
            # across two queues to halve its latency
            nc.gpsimd.dma_start(idx_sb[0:64, :], idx_d[0:64, :])
            nc.scalar.dma_start(idx_sb[64:128, :], idx_d[64:128, :])
            nc.gpsimd.memset(ones_f[:], 1.0)
            nc.vector.tensor_copy(ones_b[:], ones_f[:])

            amt_sb = const.tile([128, KE * YS], BF16, tag="amt")

            # views
            w1b8_v = w1b8_sb[:].rearrange("p (n u c) -> p n u c", n=NT, u=KE)
            w1c8_v = w1c8_sb[:].rearrange("p (n u c) -> p n u c", n=NT, u=KE)
            mts_v = mts_sb[:].rearrange("p (e m) -> p e m", e=KE)
            w1a_v = w1a_d[:].rearrange("p (q k j) -> p q k j", q=4, k=KE)
            amt_v = amt_sb[:].rearrange("p (k r) -> p k r", k=KE)

            gath_pool = ctx.enter_context(tc.tile_pool(name="gath", bufs=5))
            ygath_pool = ctx.enter_context(tc.tile_pool(name="ygath", bufs=12))
            yslab_pool = ctx.enter_context(tc.tile_pool(name="yslab", bufs=2))
            ypre_pool = ctx.enter_context(tc.tile_pool(name="ypre", bufs=2))
            tp_pool = ctx.enter_context(tc.tile_pool(name="tp", bufs=2, space="PSUM"))
            bT_pool = ctx.enter_context(tc.tile_pool(name="bT", bufs=2))
            abT_pool = ctx.enter_context(tc.tile_pool(name="abT", bufs=2))
            h_pool = ctx.enter_context(tc.tile_pool(name="h", bufs=10))
            psHA = ctx.enter_context(tc.tile_pool(name="psHA", bufs=4, space="PSUM"))
            psF = ctx.enter_context(tc.tile_pool(name="psF", bufs=2, space="PSUM"))
            o_pool = ctx.enter_context(tc.tile_pool(name="o", bufs=4))
            wa_pool = ctx.enter_context(tc.tile_pool(name="wa", bufs=4))
            taev_pool = ctx.enter_context(tc.tile_pool(name="taev", bufs=2))

            # PE warmup on the identity: keeps the p-state ramp going while
            # the first gathers and weight streams land. fill() burns PE
            # cycles into a fresh (never-read) tp tile: inserted at known
            # HBM-bound startup waits, it absorbs idle and keeps the PE
            # clock from dropping back to the 1.2 GHz p-state.
            fill_ctr = [0]

            def fill(k):
                ft = tp_pool.tile([128, E], BF16, tag="tp", space="PSUM",
                                  name=f"fill{fill_ctr[0]}")
                fill_ctr[0] += 1
                for i in range(k):
                    nc.tensor.transpose(
                        ft[:, 128 * (i % KE) : 128 * (i % KE + 1)],
                        identb[:], identb[:])

            wps = tp_pool.tile([128, E], BF16, tag="tp", name="wps", space="PSUM")
            for _ in range(16):
                nc.tensor.transpose(wps[:, 0:128], identb[:], identb[:])

            def emit_gathers(c):
                gs = []
                t0 = _RCS[c] // 128
                for t in range(_CH[c] // 128):
                    g = gath_pool.tile([128, E], BF16, tag="g", name=f"g{c}_{t}")
                    nc.gpsimd.indirect_dma_start(
                        out=g[:], out_offset=None, in_=am_d[:],
                        in_offset=bass.IndirectOffsetOnAxis(
                            ap=idx_sb[:, t0 + t : t0 + t + 1], axis=0))
                    gs.append(g)
                return gs

            def new_slabs(c):
                bT = bT_pool.tile([128, NG, 2, 512], FP8, tag="bT", name=f"bT_{c}")
                abT = abT_pool.tile([128, NG, 2, 512], FP8, tag="abT", name=f"abT_{c}")
                return bT, abT

            def transpose_unit(c, bT, abT, g_t, t, with_b):
                # 8 PE transposes into one PSUM tile, then one fp8 cast for
                # b^T slabs (alternating Scalar/DVE so neither engine gates
                # the slab supply; only for chunks on the direct-b path) +
                # one fp8 multiply (DVE) for (a*b)^T slabs.
                tp = tp_pool.tile([128, E], BF16, tag="tp", space="PSUM", name=f"tp{c}_{t}")
                for e in range(KE):
                    nc.tensor.transpose(
                        tp[:, 128 * e : 128 * (e + 1)],
                        g_t[:, 128 * e : 128 * (e + 1)],
                        identb[:],
                    )
                sl = slice(128 * t, 128 * (t + 1))
                abT_o = abT[:, :, :, sl].rearrange("p g u c -> p (g u) c")
                tp_v = tp[:].rearrange("p (e c) -> p e c", e=KE)
                if with_b:
                    bT_o = bT[:, :, :, sl].rearrange("p g u c -> p (g u) c")
                    if t % 2 == 0:
                        nc.scalar.activation(bT_o, tp_v, mybir.ActivationFunctionType.Copy)
                    else:
                        nc.vector.tensor_copy(bT_o, tp_v)
                nc.vector.tensor_tensor(
                    out=abT_o.rearrange("p e (t m) -> p e t m", m=BS),
                    in0=tp_v.rearrange("p e (t m) -> p e t m", m=BS),
                    in1=mts_v[:, :, None, :].to_broadcast([128, KE, 2, BS]),
                    op=mybir.AluOpType.mult)

            def emit_y_gathers(c):
                gs_y = []
                t0 = _RCS[c] // 128
                for t in range(_CH[c] // 128):
                    g = ygath_pool.tile([128, HID], BF16, tag="yg", name=f"yg{c}_{t}")
                    nc.gpsimd.indirect_dma_start(
                        out=g[:], out_offset=None, in_=yf_d[:],
                        in_offset=bass.IndirectOffsetOnAxis(
                            ap=idx_sb[:, t0 + t : t0 + t + 1], axis=0))
                    gs_y.append(g)
                return gs_y

            def y_pre(j):
                # this core's Y_b slice, hid tile j: (SC*W1b)^T am^T via the
                # resident fp8 weights x bf16 mention rows, cast to bf16 and
                # stored row-major for later row-gathers
                nsl = slice(128 * j, 128 * (j + 1))
                psY = psHA.tile([128, 512], F32, tag="ps_h", name=f"psY{j}")[:, 0:YS]
                for k in range(KE):
                    nc.tensor.matmul(psY[:], w1b8_v[:, j, k, :], amt_v[:, k, :],
                                     start=(k == 0), stop=(k == KE - 1))
                ye = ypre_pool.tile([128, YS], BF16, tag="ye", name=f"ye{j}")
                nc.scalar.activation(ye[:], psY[:], mybir.ActivationFunctionType.Copy)
                nc.scalar.dma_start(ys_d[:, nsl], ye[:].rearrange("p r -> r p"))

            def y_unit(c, ysl, yg_t, t):
                # 8 PE transposes of a gathered Y_b row-tile, then one copy
                # into the chunk's inject slab (Scalar/DVE alternating)
                tp = tp_pool.tile([128, HID], BF16, tag="tp", space="PSUM",
                                  name=f"ytp{c}_{t}")
                for e in range(NT):
                    nc.tensor.transpose(
                        tp[:, 128 * e : 128 * (e + 1)],
                        yg_t[:, 128 * e : 128 * (e + 1)],
                        identb[:],
                    )
                tsl = slice(128 * t, 128 * (t + 1))
                out_v = ysl[:, :, tsl]
                tp_v = tp[:].rearrange("p (n c) -> p n c", n=NT)
                if t % 2 == 1:
                    nc.scalar.activation(out_v, tp_v, mybir.ActivationFunctionType.Copy)
                else:
                    nc.vector.tensor_copy(out_v, tp_v)

            wa_tiles = [
                wa_pool.tile([128, KE, 256], BF16, tag="wa", name=f"wa{q}")
                for q in range(4)
            ]

            def phase_a(q):
                # T_a' = SC*(mentions @ W1a + b1) for wfused cols
                # 256q:256q+256 (weight stream already issued at startup)
                jsl = slice(256 * q, 256 * (q + 1))
                wa_t = wa_tiles[q]
                ps_ta = psHA.tile([128, 512], F32, tag="ps_h", name=f"ps_ta{q}")[0:BS, 0:256]
                nc.tensor.matmul(ps_ta[:], ones_b[0:1, :], b1_sb[0:1, jsl],
                                 start=True, stop=False)
                for k in range(KE):
                    nc.tensor.matmul(ps_ta[:], mts_v[:, k, :], wa_t[:, k, :],
                                     start=False, stop=(k == KE - 1))
                ev = taev_pool.tile([BS, 256], BF16, tag="taev", name=f"ev{q}")
                nc.vector.tensor_copy(ev[:], ps_ta[:])
                # Pool queue: never blocks the weight streams
                nc.gpsimd.dma_start(wfused[PW : PW + BS, jsl], ev[:])

            def w8q(qq, eng):
                sl = slice(2 * E * qq, 2 * E * (qq + 1))
                eng.dma_start(w1b8_sb[:, sl], w1b8_d[:, sl])
                eng.dma_start(w1c8_sb[:, sl], w1c8_d[:, sl])

            # ---- startup: all weight streams issue up front, spread across
            # the Sync / Scalar DGE queues so each queue's stream lands just
            # before its first consumer; gathers own the Pool queue and the
            # early HBM window, emitted two chunks ahead.
            gs = {}
            gs[0] = emit_gathers(0)
            cur = new_slabs(0)
            transpose_unit(0, cur[0], cur[1], gs[0][0], 0, True)
            nc.scalar.dma_start(wa_tiles[0][:], w1a_v[:, 0])
            if USE_YB:
                nc.scalar.dma_start(amt_sb[:], amt_d[:])
            for q in range(1, 4):
                nc.scalar.dma_start(wa_tiles[q][:], w1a_v[:, q])
            w8q(0, nc.sync)
            nc.sync.dma_start(pwS_sb[:, 0:512], pwS_d[:, 0:512])
            w8q(1, nc.sync)
            w8q(2, nc.sync)
            w8q(3, nc.scalar)
            nc.sync.dma_start(pwS_sb[:, 512:R], pwS_d[:, 512:R])
            phase_a(0)
            for t in range(1, _CH[0] // 128):
                transpose_unit(0, cur[0], cur[1], gs[0][t], t, True)

            # ---- main loop
            w2q = []  # pipelined (h_t, n, NC, ps_f, c) entries

            def emit_out(ps_fp, cc):
                # chunk's fine scores complete in ps_f: add rough, store
                rcc, NCc = _RCS[cc], _CH[cc]
                o_t = o_pool.tile([1, 512], F32, tag="o", name=f"o{cc}")
                nc.vector.tensor_tensor(out=o_t[0:1, :NCc], in0=ps_fp[0:1, :NCc],
                                        in1=rough_sb[0:1, rcc : rcc + NCc],
                                        op=mybir.AluOpType.add)
                nc.sync.dma_start(out_d[0:1, rcc : rcc + NCc], o_t[0:1, :NCc])

            def flush_w2(limit):
                while len(w2q) > limit:
                    h_t, n, NCp, ps_fp, cc = w2q.pop(0)
                    nc.tensor.matmul(ps_fp[0:1, :NCp], w2_sb[:, n : n + 1], h_t[:, :NCp],
                                     start=(n == 0), stop=(n == NT - 1))
                    if n == NT - 1:
                        emit_out(ps_fp, cc)

            ygs = {}
            ycur = None
            for c in range(NCHUNK):
                rc = _RCS[c]
                NC = _CH[c]
                bT, abT = cur
                if c + 1 < NCHUNK:
                    gs[c + 1] = emit_gathers(c + 1)
                if c >= YC0 and c + 2 < NCHUNK:
                    ygs[c + 2] = emit_y_gathers(c + 2)
                if c + 1 < NCHUNK:
                    nxt = new_slabs(c + 1)
                    ysl_nxt = (
                        yslab_pool.tile([128, NT, 512], BF16, tag="ysl",
                                        name=f"ysl{c + 1}")
                        if c + 1 >= YC0 else None)
                    units = []
                    for t in range(_CH[c + 1] // 128):
                        units.append(("am", t))
                        if c + 1 >= YC0:
                            units.append(("y", t))
                else:
                    nxt, ysl_nxt, units = None, None, []
                per_group = (len(units) + NT - 1) // NT if units else 0

                ps_f = psF.tile([1, 512], F32, tag="ps_f", name=f"ps_f{c}")
                for n in range(NT):
                    if c == 0 and n == 0:
                        phase_a(1)
                    if c == 0 and n == 2:
                        phase_a(2)
                    if c == 0 and n == 4:
                        phase_a(3)
                    nsl = slice(128 * n, 128 * (n + 1))
                    ps_h = psHA.tile([128, 512], F32, tag="ps_h", name=f"ps_h{c}_{n}")
                    # One full-width accumulation block per unit. The b-term
                    # is 4 DoubleRow passes for early chunks; once the
                    # all-gathered Y_b = SC*(all_mentions @ W1b) is ready it
                    # collapses to a single identity-inject of the gathered
                    # hidden rows (2048 -> 512 stream cycles per unit).
                    if c >= YC0:
                        nc.tensor.matmul(
                            ps_h[:, 0:NC], identb[:], ycur[:, n, 0:NC],
                            start=True, stop=False)
                    else:
                        for g in range(NG):
                            nc.tensor.matmul(
                                ps_h[:, 0:NC], w1b8_v[:, n, 2 * g : 2 * g + 2, :],
                                bT[:, g, :, 0:NC],
                                start=(g == 0), stop=False, perf_mode=DRM)
                    for g in range(NG):
                        nc.tensor.matmul(
                            ps_h[:, 0:NC], w1c8_v[:, n, 2 * g : 2 * g + 2, :],
                            abT[:, g, :, 0:NC],
                            start=False, stop=False, perf_mode=DRM)
                    nc.tensor.matmul(
                        ps_h[:, 0:NC], wfused[:, nsl],
                        pwS_sb[:, rc : rc + NC],
                        start=False, stop=True)
                    h_t = h_pool.tile([128, 512], BF16, tag="h", name=f"h{c}_{n}")
                    nc.scalar.activation(h_t[:, :NC], ps_h[:, :NC],
                                         mybir.ActivationFunctionType.Lrelu, alpha=ALPHA)
                    # w2 batches at chunk end (fewer fp8<->bf16 weight-mode
                    # flips in the PE stream), two entries deferred into the
                    # next chunk so PE never waits on this chunk's last Lrelu
                    w2q.append((h_t, n, NC, ps_f, c))
                    for _ in range(per_group):
                        if units:
                            kind, t = units.pop(0)
                            if kind == "am":
                                transpose_unit(c + 1, nxt[0], nxt[1],
                                               gs[c + 1][t], t, c + 1 < YC0)
                            else:
                                y_unit(c + 1, ysl_nxt, ygs[c + 1][t], t)
                    if c == 0 and USE_YB:
                        y_pre(n)
                    # partial mid-chunk flush: keeps the end-of-chunk w2
                    # batch shallow enough that its deepest pops never wait
                    # on a just-issued Lrelu
                    if n == 4:
                        flush_w2(4)
                flush_w2(2)
                if c == 0 and USE_YB:
                    # per-core Y_b slices are all stored: gather the full
                    # table, then queue the first Y row-gathers behind it
                    nc.gpsimd.collective_compute(
                        "AllGather", mybir.AluOpType.bypass,
                        replica_groups=[[i for i in range(NCORES)]],
                        ins=[ys_d[:]], outs=[yf_d[:]])
                    ygs[YC0] = emit_y_gathers(YC0)
                    ygs[YC0 + 1] = emit_y_gathers(YC0 + 1)
                cur = nxt
                ycur = ysl_nxt
                gs.pop(c, None)
                ygs.pop(c, None)
            flush_w2(0)

    _redistribute_waits(nc, helper_sems)
    return nc



_NC_CACHE = None


def _get_nc():
    global _NC_CACHE
    if _NC_CACHE is None:
        _NC_CACHE = build_nc()
    return _NC_CACHE


def make_in_maps(
    all_mentions,
    mentions_batch,
    pw_batch,
    top_indices_batch,
    top_rough_scores_batch,
    W1,
    b1,
    W2,
    b2,
):
    am = np.asarray(all_mentions, np.float32)
    men = np.asarray(mentions_batch, np.float32)
    pw = np.asarray(pw_batch, np.float32)
    idx = np.asarray(top_indices_batch).astype(np.int32)
    rough = np.asarray(top_rough_scores_batch, np.float32)
    W1 = np.asarray(W1, np.float32)
    b1 = np.asarray(b1, np.float32)
    W2 = np.asarray(W2, np.float32)
    b2 = np.asarray(b2, np.float32)

    am_bf = np.ascontiguousarray(am.astype(BF_NP))

    # [p, n, kt, j] = e4m3(SC * W1x[128*kt + p, 128*n + j])
    def w8(Wx):
        w = (SC * Wx).reshape(KE, 128, NT, 128)  # [kt, p, n, j]
        w = w.transpose(1, 2, 0, 3).reshape(128, NT * E)
        return np.ascontiguousarray(w.astype(F8_NP))

    w1b8 = w8(W1[E : 2 * E])
    w1c8 = w8(W1[2 * E : 3 * E])
    # [p, q, kt, jj] = bf16(SC * W1a[128*kt + p, 256*q + jj])
    w1a = (SC * W1[0:E]).reshape(KE, 128, 4, 256).transpose(1, 2, 0, 3).reshape(128, KE * HID)
    w1a = np.ascontiguousarray(w1a.astype(BF_NP))
    w1d = np.ascontiguousarray((SC * W1[3 * E : 3 * E + PW]).astype(BF_NP))
    w2r = np.ascontiguousarray((W2[:, 0] / SC).reshape(NT, 128).T.astype(BF_NP))
    b1r = np.ascontiguousarray((SC * b1).reshape(1, HID).astype(BF_NP))
    S = np.tile(np.eye(BS, dtype=np.float32), (1, A))

    in_maps = []
    for c in range(NCORES):
        sl = slice(c * BS, (c + 1) * BS)
        # mts[p, e, m] = bf16(men[c*BS + m, 128*e + p])
        mts = men[sl].T.reshape(KE, 128, BS).transpose(1, 0, 2).reshape(128, KE * BS)
        mts = np.ascontiguousarray(mts.astype(BF_NP))
        # amt[p, k, r] = bf16(am[c*YS + r, 128*k + p]) — this core's
        # Y_b precompute slice, k-tiled transposed
        ysl = slice(c * YS, (c + 1) * YS)
        amt = am[ysl].T.reshape(KE, 128, YS).transpose(1, 0, 2).reshape(128, KE * YS)
        amt = np.ascontiguousarray(amt.astype(BF_NP))
        pwT = pw[sl].transpose(2, 1, 0).reshape(PW, R)
        pwS = np.ascontiguousarray(np.concatenate([pwT, S], axis=0).astype(BF_NP))
        idx_r = np.ascontiguousarray(idx[sl].T.reshape(R // 128, 128).T)
        # b2 is a constant scalar on every score: fold it in here (exact)
        rough_r = np.ascontiguousarray(rough[sl].T.reshape(1, R) + np.float32(b2[0]))
        in_maps.append(
            dict(
                am=am_bf,
                amt=amt,
                idb=np.eye(128, dtype=BF_NP),
                mts=mts,
                pwS=pwS,
                idx=idx_r,
                rough=rough_r,
                w1a=w1a,
                w1b8=w1b8,
                w1c8=w1c8,
                w1d=w1d,
                w2r=w2r,
                b1r=b1r,
            )
        )
    return in_maps


def assemble_output(results):
    scores = np.empty((BATCH, A), np.float32)
    for c in range(NCORES):
        score_r = np.asarray(results[c]["out"]).reshape(A, BS)
        scores[c * BS : (c + 1) * BS, :] = score_r.T
    out = np.empty((BATCH, A + 1), np.float32)
    out[:, 0] = EPSILON
    out[:, 1:] = scores
    return out


def kernel(**inputs):
    nc = _get_nc()
    in_maps = make_in_maps(**inputs)
    res = run_bass_kernel_spmd(nc, in_maps, core_ids=list(range(NCORES)))
    return assemble_output(res.results)


if __name__ == "__main__":
    nc = build_nc()
    print("built ok")



# revision 50
# speedup vs baseline: 1.0268x; 1.0268x over previous
"""Trainium2 Bass kernel for the AnaphoricityScorer problem.

Data-parallel over the batch (mention) dimension across 8 NeuronCores.
Per core: 64 mentions x 50 antecedents = 3200 pair rows, r = ant*64 + m.

pair = [a, b, a*b, pw] @ W1 with the a-term folded into a fused k-tile
(T_a' = mentions @ W1a + b1 injected through a 0/1 selection matrix S).
The b and a*b terms run as fp8(e4m3) DoubleRow matmuls (K=256 per
instruction at 1 output row/cycle, 2x bf16 throughput); to condition
the fp8 weights, the whole pre-activation is scaled by 64 (W1*64,
b1*64 exact power-of-2 scalings) and undone through W2/64 - valid
because LeakyReLU is positively homogeneous. Gathers, transposes and
the fused tile run in bf16. Gathered rows transpose on the
TensorEngine (bf16, 1 cycle/row) into PSUM; per 128-row tile one fp8
cast (alternating Scalar/DVE) makes the b^T slabs and one DVE multiply
makes the fp8 (a*b)^T slabs, laid out [128, 2, N] for DoubleRow
consumption.

Row chunks of [256, 256, 512x5, 128] accumulate full-width in one PSUM
bank per (hid-tile, chunk) unit so each DoubleRow LDWEIGHTS (256
weight cols ~ 256 cycles) feeds a full 512-row moving stream. The w2
reduction batches at chunk end with two entries deferred into the next
chunk (emit_out rides the flush), keeping the PE off the Lrelu
critical path and minimizing fp8<->bf16 weight-mode flips. Startup
streams are spread across the Sync (fp8 weights + pwS) and Scalar
(idx + W1a quarters) DGE queues while indirect gathers own the Pool
queue. USE_YB gates an experimental AllGather-based Y_b=am@W1b
precompute path (disabled: NEFF load fails; see session notes).
"""

import os
import sys
from contextlib import ExitStack

import numpy as np
import ml_dtypes

for _p in ("/opt/trn_rl_repo",):
    if _p not in sys.path and os.path.isdir(_p):
        sys.path.insert(0, _p)

from concourse import bass, mybir  # noqa: E402
import concourse.tile as tile  # noqa: E402
from concourse.masks import make_identity  # noqa: E402
from concourse.bass_utils import run_bass_kernel_spmd  # noqa: E402

NM, BATCH, A, E, PW, HID, NCORES = 2000, 512, 50, 1024, 64, 1024, 8
BS = BATCH // NCORES
R = A * BS
USE_YB = False  # Y_b AllGather inject path (crashes NEFF load; see memory)
YS = NM // NCORES  # Y_b precompute rows per core
YC0 = 3 if USE_YB else 10**9  # first chunk using the Y_b inject path
ALPHA, EPSILON = 0.01, 1e-07
SC = 64.0
F32 = mybir.dt.float32
BF16 = mybir.dt.bfloat16
FP8 = mybir.dt.float8e4
I32 = mybir.dt.int32
DRM = mybir.MatmulPerfMode.DoubleRow
KE, NT, NG = E // 128, HID // 128, E // 256

_CH = [256, 256, 512, 512, 512, 512, 512, 128]
_RCS = [0, 256, 512, 1024, 1536, 2048, 2560, 3072]
NCHUNK = len(_CH)

BF_NP = ml_dtypes.bfloat16
F8_NP = ml_dtypes.float8_e4m3


def _redistribute_waits(nc, helper_sems, limit=1):
    """Enforce <=1 sync wait per instruction (walrus limit on this build).

    Compute-engine instructions execute in-order on their engine stream, so
    excess waits hoist into single-wait InstEventSemaphore instructions
    spliced just before them. DMACopy instructions execute from concurrent
    DGE queue programs, so an engine-stream EventSem does NOT gate them:
    their waits are bridged through a per-engine helper semaphore - the
    EventSems consume the original waits on the engine stream and increment
    the helper; the DMA's single wait slot watches the helper's cumulative
    count. Helpers are decremented back to zero at the end so repeated
    executions of the loaded NEFF stay correct.
    """
    counter = [0]
    counts = {e: 0 for e in helper_sems}
    last_dma = {}
    last_blk = None

    def mk_ev(engine, wait=None, update=None):
        ev = mybir.InstEventSemaphore(
            name=f"hoistw-{counter[0]}", ins=[], outs=[]
        )
        counter[0] += 1
        ev.engine = engine
        ev.sync_info = mybir.SyncInfo(
            on_wait=[wait] if wait else [], on_update=[update] if update else []
        )
        return ev

    for f in nc.m.functions:
        for blk in f.blocks:
            il = blk.instructions
            if il:
                last_blk = blk
            new_il = []
            changed = False
            for inst in il:
                si = inst.sync_info
                waits = list(si.on_wait) if si is not None else []
                if isinstance(inst, mybir.InstDMACopy) and len(waits) > limit:
                    h = helper_sems[inst.engine]
                    for i, w in enumerate(waits):
                        upd = None
                        if i == len(waits) - 1:
                            upd = mybir.SyncUpdate(
                                sync_type="semaphore",
                                id=h.num,
                                ant_name=h.name,
                                update_mode="sem-inc",
                                update_value=1,
                            )
                        new_il.append(mk_ev(inst.engine, w, upd))
                    counts[inst.engine] += 1
                    last_dma[inst.engine] = inst
                    si.on_wait = [
                        mybir.SyncWait(
                            sync_type="semaphore",
                            id=h.num,
                            ant_name=h.name,
                            wait_mode="sem-ge-imm",
                            wait_value=counts[inst.engine],
                        )
                    ]
                    changed = True
                elif len(waits) > limit:
                    for w in waits[:-limit]:
                        new_il.append(mk_ev(inst.engine, w))
                    si.on_wait = waits[-limit:]
                    changed = True
                new_il.append(inst)
            if changed:
                blk.instructions = new_il

    # Reset each helper to zero right AFTER that engine's last hoisted DMA
    # (cumulative thresholds make earlier placement unsafe, and
    # end-of-stream placement would serialize the reset into the kernel
    # tail). A single write-immediate replaces the old N-deep decrement
    # chain, which serialized ~100ns/instruction into the tail barrier.
    def decs(eng, h, n):
        return [
            mk_ev(
                eng,
                None,
                mybir.SyncUpdate(
                    sync_type="semaphore",
                    id=h.num,
                    ant_name=h.name,
                    update_mode="sem-wr-imm",
                    update_value=0,
                ),
            )
        ]

    for f in nc.m.functions:
        for blk in f.blocks:
            il = blk.instructions
            new_il = []
            changed = False
            for inst in il:
                new_il.append(inst)
                for eng, h in helper_sems.items():
                    if counts[eng] and last_dma.get(eng) is inst:
                        new_il.extend(decs(eng, h, counts[eng]))
                        changed = True
            if changed:
                blk.instructions = new_il


def build_nc():
    nc = bass.Bass("TRN2", target_bir_lowering=False, debug=False,
                   num_devices=NCORES)
    am_d = nc.declare_dram_parameter("am", [NM, E], BF16, isOutput=False)
    amt_d = nc.declare_dram_parameter("amt", [128, KE * YS], BF16, isOutput=False)
    ys_d = nc.dram_tensor("ysl", [YS, HID], BF16, kind="Internal")
    yf_d = nc.dram_tensor("yfl", [NM, HID], BF16, kind="Internal")
    id_d = nc.declare_dram_parameter("idb", [128, 128], BF16, isOutput=False)
    mts_d = nc.declare_dram_parameter("mts", [128, KE * BS], BF16, isOutput=False)
    pwS_d = nc.declare_dram_parameter("pwS", [128, R], BF16, isOutput=False)
    idx_d = nc.declare_dram_parameter("idx", [128, R // 128], I32, isOutput=False)
    rough_d = nc.declare_dram_parameter("rough", [1, R], F32, isOutput=False)
    w1a_d = nc.declare_dram_parameter("w1a", [128, KE * HID], BF16, isOutput=False)
    w1b8_d = nc.declare_dram_parameter("w1b8", [128, NT * E], FP8, isOutput=False)
    w1c8_d = nc.declare_dram_parameter("w1c8", [128, NT * E], FP8, isOutput=False)
    w1d_d = nc.declare_dram_parameter("w1d", [PW, HID], BF16, isOutput=False)
    w2r_d = nc.declare_dram_parameter("w2r", [128, NT], BF16, isOutput=False)
    b1r_d = nc.declare_dram_parameter("b1r", [1, HID], BF16, isOutput=False)
    out_d = nc.declare_dram_parameter("out", [1, R], F32, isOutput=True)

    helper_sems = {
        mybir.EngineType.SP: nc.alloc_semaphore("hoist_dma_sp"),
        mybir.EngineType.Pool: nc.alloc_semaphore("hoist_dma_pool"),
        mybir.EngineType.Activation: nc.alloc_semaphore("hoist_dma_act"),
    }

    with tile.TileContext(nc) as tc:
        with ExitStack() as ctx:
            const = ctx.enter_context(tc.tile_pool(name="const", bufs=1))
            w1b8_sb = const.tile([128, NT * E], FP8, tag="w1b8")
            w1c8_sb = const.tile([128, NT * E], FP8, tag="w1c8")
            wfused = const.tile([128, HID], BF16, tag="wfused")
            pwS_sb = const.tile([128, R], BF16, tag="pwS")
            mts_sb = const.tile([128, KE * BS], BF16, tag="mts")
            w2_sb = const.tile([128, NT], BF16, tag="w2")
            b1_sb = const.tile([1, HID], BF16, tag="b1")
            identb = const.tile([128, 128], BF16, tag="identb")
            idx_sb = const.tile([128, R // 128], I32, tag="idx")
            rough_sb = const.tile([1, R], F32, tag="rough")
            ones_f = const.tile([1, BS], F32, tag="ones_f")
            ones_b = const.tile([1, BS], BF16, tag="ones_b")

            # small consts first on the SP DMA queue
            nc.sync.dma_start(identb[:], id_d[:])
            nc.sync.dma_start(b1_sb[:], b1r_d[:])
            nc.sync.dma_start(mts_sb[:], mts_d[:])
            nc.sync.dma_start(w2_sb[:], w2r_d[:])
            nc.sync.dma_start(wfused[0:PW, :], w1d_d[:])
            nc.sync.dma_start(rough_sb[:], rough_d[:])
            # idx is a [128, 25] strided write: descriptor-bound, so split
            # across two queues to halve its latency
            nc.gpsimd.dma_start(idx_sb[0:64, :], idx_d[0:64, :])
            nc.scalar.dma_start(idx_sb[64:128, :], idx_d[64:128, :])
            nc.gpsimd.memset(ones_f[:], 1.0)
            nc.vector.tensor_copy(ones_b[:], ones_f[:])

            amt_sb = const.tile([128, KE * YS], BF16, tag="amt")

            # views
            w1b8_v = w1b8_sb[:].rearrange("p (n u c) -> p n u c", n=NT, u=KE)
            w1c8_v = w1c8_sb[:].rearrange("p (n u c) -> p n u c", n=NT, u=KE)
            mts_v = mts_sb[:].rearrange("p (e m) -> p e m", e=KE)
            w1a_v = w1a_d[:].rearrange("p (q k j) -> p q k j", q=4, k=KE)
            amt_v = amt_sb[:].rearrange("p (k r) -> p k r", k=KE)

            gath_pool = ctx.enter_context(tc.tile_pool(name="gath", bufs=5))
            ygath_pool = ctx.enter_context(tc.tile_pool(name="ygath", bufs=12))
            yslab_pool = ctx.enter_context(tc.tile_pool(name="yslab", bufs=2))
            ypre_pool = ctx.enter_context(tc.tile_pool(name="ypre", bufs=2))
            tp_pool = ctx.enter_context(tc.tile_pool(name="tp", bufs=2, space="PSUM"))
            bT_pool = ctx.enter_context(tc.tile_pool(name="bT", bufs=2))
            abT_pool = ctx.enter_context(tc.tile_pool(name="abT", bufs=2))
            h_pool = ctx.enter_context(tc.tile_pool(name="h", bufs=10))
            psHA = ctx.enter_context(tc.tile_pool(name="psHA", bufs=4, space="PSUM"))
            psF = ctx.enter_context(tc.tile_pool(name="psF", bufs=2, space="PSUM"))
            o_pool = ctx.enter_context(tc.tile_pool(name="o", bufs=4))
            wa_pool = ctx.enter_context(tc.tile_pool(name="wa", bufs=4))
            taev_pool = ctx.enter_context(tc.tile_pool(name="taev", bufs=2))

            # PE warmup on the identity: keeps the p-state ramp going while
            # the first gathers and weight streams land. fill() burns PE
            # cycles into a fresh (never-read) tp tile: inserted at known
            # HBM-bound startup waits, it absorbs idle and keeps the PE
            # clock from dropping back to the 1.2 GHz p-state.
            fill_ctr = [0]

            def fill(k):
                ft = tp_pool.tile([128, E], BF16, tag="tp", space="PSUM",
                                  name=f"fill{fill_ctr[0]}")
                fill_ctr[0] += 1
                for i in range(k):
                    nc.tensor.transpose(
                        ft[:, 128 * (i % KE) : 128 * (i % KE + 1)],
                        identb[:], identb[:])

            wps = tp_pool.tile([128, E], BF16, tag="tp", name="wps", space="PSUM")
            for _ in range(16):
                nc.tensor.transpose(wps[:, 0:128], identb[:], identb[:])

            def emit_gathers(c):
                gs = []
                t0 = _RCS[c] // 128
                for t in range(_CH[c] // 128):
                    g = gath_pool.tile([128, E], BF16, tag="g", name=f"g{c}_{t}")
                    nc.gpsimd.indirect_dma_start(
                        out=g[:], out_offset=None, in_=am_d[:],
                        in_offset=bass.IndirectOffsetOnAxis(
                            ap=idx_sb[:, t0 + t : t0 + t + 1], axis=0))
                    gs.append(g)
                return gs

            def new_slabs(c):
                bT = bT_pool.tile([128, NG, 2, 512], FP8, tag="bT", name=f"bT_{c}")
                abT = abT_pool.tile([128, NG, 2, 512], FP8, tag="abT", name=f"abT_{c}")
                return bT, abT

            def transpose_unit(c, bT, abT, g_t, t, with_b):
                # 8 PE transposes into one PSUM tile, then one fp8 cast for
                # b^T slabs (alternating Scalar/DVE so neither engine gates
                # the slab supply; only for chunks on the direct-b path) +
                # one fp8 multiply (DVE) for (a*b)^T slabs.
                tp = tp_pool.tile([128, E], BF16, tag="tp", space="PSUM", name=f"tp{c}_{t}")
                for e in range(KE):
                    nc.tensor.transpose(
                        tp[:, 128 * e : 128 * (e + 1)],
                        g_t[:, 128 * e : 128 * (e + 1)],
                        identb[:],
                    )
                sl = slice(128 * t, 128 * (t + 1))
                abT_o = abT[:, :, :, sl].rearrange("p g u c -> p (g u) c")
                tp_v = tp[:].rearrange("p (e c) -> p e c", e=KE)
                if with_b:
                    bT_o = bT[:, :, :, sl].rearrange("p g u c -> p (g u) c")
                    if t % 2 == 0:
                        nc.scalar.activation(bT_o, tp_v, mybir.ActivationFunctionType.Copy)
                    else:
                        nc.vector.tensor_copy(bT_o, tp_v)
                nc.vector.tensor_tensor(
                    out=abT_o.rearrange("p e (t m) -> p e t m", m=BS),
                    in0=tp_v.rearrange("p e (t m) -> p e t m", m=BS),
                    in1=mts_v[:, :, None, :].to_broadcast([128, KE, 2, BS]),
                    op=mybir.AluOpType.mult)

            def emit_y_gathers(c):
                gs_y = []
                t0 = _RCS[c] // 128
                for t in range(_CH[c] // 128):
                    g = ygath_pool.tile([128, HID], BF16, tag="yg", name=f"yg{c}_{t}")
                    nc.gpsimd.indirect_dma_start(
                        out=g[:], out_offset=None, in_=yf_d[:],
                        in_offset=bass.IndirectOffsetOnAxis(
                            ap=idx_sb[:, t0 + t : t0 + t + 1], axis=0))
                    gs_y.append(g)
                return gs_y

            def y_pre(j):
                # this core's Y_b slice, hid tile j: (SC*W1b)^T am^T via the
                # resident fp8 weights x bf16 mention rows, cast to bf16 and
                # stored row-major for later row-gathers
                nsl = slice(128 * j, 128 * (j + 1))
                psY = psHA.tile([128, 512], F32, tag="ps_h", name=f"psY{j}")[:, 0:YS]
                for k in range(KE):
                    nc.tensor.matmul(psY[:], w1b8_v[:, j, k, :], amt_v[:, k, :],
                                     start=(k == 0), stop=(k == KE - 1))
                ye = ypre_pool.tile([128, YS], BF16, tag="ye", name=f"ye{j}")
                nc.scalar.activation(ye[:], psY[:], mybir.ActivationFunctionType.Copy)
                nc.scalar.dma_start(ys_d[:, nsl], ye[:].rearrange("p r -> r p"))

            def y_unit(c, ysl, yg_t, t):
                # 8 PE transposes of a gathered Y_b row-tile, then one copy
                # into the chunk's inject slab (Scalar/DVE alternating)
                tp = tp_pool.tile([128, HID], BF16, tag="tp", space="PSUM",
                                  name=f"ytp{c}_{t}")
                for e in range(NT):
                    nc.tensor.transpose(
                        tp[:, 128 * e : 128 * (e + 1)],
                        yg_t[:, 128 * e : 128 * (e + 1)],
                        identb[:],
                    )
                tsl = slice(128 * t, 128 * (t + 1))
                out_v = ysl[:, :, tsl]
                tp_v = tp[:].rearrange("p (n c) -> p n c", n=NT)
                if t % 2 == 1:
                    nc.scalar.activation(out_v, tp_v, mybir.ActivationFunctionType.Copy)
                else:
                    nc.vector.tensor_copy(out_v, tp_v)

            wa_tiles = [
                wa_pool.tile([128, KE, 256], BF16, tag="wa", name=f"wa{q}")
                for q in range(4)
            ]

            def phase_a(q):
                # T_a' = SC*(mentions @ W1a + b1) for wfused cols
                # 256q:256q+256 (weight stream already issued at startup)
                jsl = slice(256 * q, 256 * (q + 1))
                wa_t = wa_tiles[q]
                ps_ta = psHA.tile([128, 512], F32, tag="ps_h", name=f"ps_ta{q}")[0:BS, 0:256]
                nc.tensor.matmul(ps_ta[:], ones_b[0:1, :], b1_sb[0:1, jsl],
                                 start=True, stop=False)
                for k in range(KE):
                    nc.tensor.matmul(ps_ta[:], mts_v[:, k, :], wa_t[:, k, :],
                                     start=False, stop=(k == KE - 1))
                ev = taev_pool.tile([BS, 256], BF16, tag="taev", name=f"ev{q}")
                nc.vector.tensor_copy(ev[:], ps_ta[:])
                # Pool queue: never blocks the weight streams
                nc.gpsimd.dma_start(wfused[PW : PW + BS, jsl], ev[:])

            def w8q(qq, eng):
                sl = slice(2 * E * qq, 2 * E * (qq + 1))
                eng.dma_start(w1b8_sb[:, sl], w1b8_d[:, sl])
                eng.dma_start(w1c8_sb[:, sl], w1c8_d[:, sl])

            # ---- startup: all weight streams issue up front, spread across
            # the Sync / Scalar DGE queues so each queue's stream lands just
            # before its first consumer; gathers own the Pool queue and the
            # early HBM window, emitted two chunks ahead.
            gs = {}
            gs[0] = emit_gathers(0)
            cur = new_slabs(0)
            transpose_unit(0, cur[0], cur[1], gs[0][0], 0, True)
            nc.scalar.dma_start(wa_tiles[0][:], w1a_v[:, 0])
            if USE_YB:
                nc.scalar.dma_start(amt_sb[:], amt_d[:])
            for q in range(1, 4):
                nc.scalar.dma_start(wa_tiles[q][:], w1a_v[:, q])
            w8q(0, nc.sync)
            nc.sync.dma_start(pwS_sb[:, 0:512], pwS_d[:, 0:512])
            w8q(1, nc.sync)
            w8q(2, nc.sync)
            w8q(3, nc.scalar)
            nc.sync.dma_start(pwS_sb[:, 512:R], pwS_d[:, 512:R])
            phase_a(0)
            for t in range(1, _CH[0] // 128):
                transpose_unit(0, cur[0], cur[1], gs[0][t], t, True)

            # ---- main loop
            w2q = []  # pipelined (h_t, n, NC, ps_f, c) entries

            def emit_out(ps_fp, cc):
                # chunk's fine scores complete in ps_f: add rough, store
                rcc, NCc = _RCS[cc], _CH[cc]
                o_t = o_pool.tile([1, 512], F32, tag="o", name=f"o{cc}")
                nc.vector.tensor_tensor(out=o_t[0:1, :NCc], in0=ps_fp[0:1, :NCc],
                                        in1=rough_sb[0:1, rcc : rcc + NCc],
                                        op=mybir.AluOpType.add)
                nc.sync.dma_start(out_d[0:1, rcc : rcc + NCc], o_t[0:1, :NCc])

            def flush_w2(limit):
                while len(w2q) > limit:
                    h_t, n, NCp, ps_fp, cc = w2q.pop(0)
                    nc.tensor.matmul(ps_fp[0:1, :NCp], w2_sb[:, n : n + 1], h_t[:, :NCp],
                                     start=(n == 0), stop=(n == NT - 1))
                    if n == NT - 1:
                        emit_out(ps_fp, cc)

            ygs = {}
            ycur = None
            for c in range(NCHUNK):
                rc = _RCS[c]
                NC = _CH[c]
                bT, abT = cur
                if c + 1 < NCHUNK:
                    gs[c + 1] = emit_gathers(c + 1)
                if c >= YC0 and c + 2 < NCHUNK:
                    ygs[c + 2] = emit_y_gathers(c + 2)
                if c + 1 < NCHUNK:
                    nxt = new_slabs(c + 1)
                    ysl_nxt = (
                        yslab_pool.tile([128, NT, 512], BF16, tag="ysl",
                                        name=f"ysl{c + 1}")
                        if c + 1 >= YC0 else None)
                    units = []
                    for t in range(_CH[c + 1] // 128):
                        units.append(("am", t))
                        if c + 1 >= YC0:
                            units.append(("y", t))
                else:
                    nxt, ysl_nxt, units = None, None, []
                per_group = (len(units) + NT - 1) // NT if units else 0

                ps_f = psF.tile([1, 512], F32, tag="ps_f", name=f"ps_f{c}")
                for n in range(NT):
                    if c == 0 and n == 0:
                        phase_a(1)
                    if c == 0 and n == 2:
                        phase_a(2)
                    if c == 0 and n == 4:
                        phase_a(3)
                    nsl = slice(128 * n, 128 * (n + 1))
                    ps_h = psHA.tile([128, 512], F32, tag="ps_h", name=f"ps_h{c}_{n}")
                    # One full-width accumulation block per unit. The b-term
                    # is 4 DoubleRow passes for early chunks; once the
                    # all-gathered Y_b = SC*(all_mentions @ W1b) is ready it
                    # collapses to a single identity-inject of the gathered
                    # hidden rows (2048 -> 512 stream cycles per unit).
                    if c >= YC0:
                        nc.tensor.matmul(
                            ps_h[:, 0:NC], identb[:], ycur[:, n, 0:NC],
                            start=True, stop=False)
                    else:
                        for g in range(NG):
                            nc.tensor.matmul(
                                ps_h[:, 0:NC], w1b8_v[:, n, 2 * g : 2 * g + 2, :],
                                bT[:, g, :, 0:NC],
                                start=(g == 0), stop=False, perf_mode=DRM)
                    for g in range(NG):
                        nc.tensor.matmul(
                            ps_h[:, 0:NC], w1c8_v[:, n, 2 * g : 2 * g + 2, :],
                            abT[:, g, :, 0:NC],
                            start=False, stop=False, perf_mode=DRM)
                    nc.tensor.matmul(
                        ps_h[:, 0:NC], wfused[:, nsl],
                        pwS_sb[:, rc : rc + NC],
                        start=False, stop=True)
                    h_t = h_pool.tile([128, 512], BF16, tag="h", name=f"h{c}_{n}")
                    nc.scalar.activation(h_t[:, :NC], ps_h[:, :NC],
                                         mybir.ActivationFunctionType.Lrelu, alpha=ALPHA)
                    # w2 batches at chunk end (fewer fp8<->bf16 weight-mode
                    # flips in the PE stream), two entries deferred into the
                    # next chunk so PE never waits on this chunk's last Lrelu
                    w2q.append((h_t, n, NC, ps_f, c))
                    for _ in range(per_group):
                        if units:
                            kind, t = units.pop(0)
                            if kind == "am":
                                transpose_unit(c + 1, nxt[0], nxt[1],
                                               gs[c + 1][t], t, c + 1 < YC0)
                            else:
                                y_unit(c + 1, ysl_nxt, ygs[c + 1][t], t)
                    if c == 0 and USE_YB:
                        y_pre(n)
                flush_w2(2)
                if c == 0 and USE_YB:
                    # per-core Y_b slices are all stored: gather the full
                    # table, then queue the first Y row-gathers behind it
                    nc.gpsimd.collective_compute(
                        "AllGather", mybir.AluOpType.bypass,
                        replica_groups=[[i for i in range(NCORES)]],
                        ins=[ys_d[:]], outs=[yf_d[:]])
                    ygs[YC0] = emit_y_gathers(YC0)
                    ygs[YC0 + 1] = emit_y_gathers(YC0 + 1)
                cur = nxt
                ycur = ysl_nxt
                gs.pop(c, None)
                ygs.pop(c, None)
            flush_w2(0)

    _redistribute_waits(nc, helper_sems)
    return nc



_NC_CACHE = None


def _get_nc():
    global _NC_CACHE
    if _NC_CACHE is None:
        _NC_CACHE = build_nc()
    return _NC_CACHE


def make_in_maps(
    all_mentions,
    mentions_batch,
    pw_batch,
    top_indices_batch,
    top_rough_scores_batch,
    W1,
    b1,
    W2,
    b2,
):
    am = np.asarray(all_mentions, np.float32)
    men = np.asarray(mentions_batch, np.float32)
    pw = np.asarray(pw_batch, np.float32)
    idx = np.asarray(top_indices_batch).astype(np.int32)
    rough = np.asarray(top_rough_scores_batch, np.float32)
    W1 = np.asarray(W1, np.float32)
    b1 = np.asarray(b1, np.float32)
    W2 = np.asarray(W2, np.float32)
    b2 = np.asarray(b2, np.float32)

    am_bf = np.ascontiguousarray(am.astype(BF_NP))

    # [p, n, kt, j] = e4m3(SC * W1x[128*kt + p, 128*n + j])
    def w8(Wx):
        w = (SC * Wx).reshape(KE, 128, NT, 128)  # [kt, p, n, j]
        w = w.transpose(1, 2, 0, 3).reshape(128, NT * E)
        return np.ascontiguousarray(w.astype(F8_NP))

    w1b8 = w8(W1[E : 2 * E])
    w1c8 = w8(W1[2 * E : 3 * E])
    # [p, q, kt, jj] = bf16(SC * W1a[128*kt + p, 256*q + jj])
    w1a = (SC * W1[0:E]).reshape(KE, 128, 4, 256).transpose(1, 2, 0, 3).reshape(128, KE * HID)
    w1a = np.ascontiguousarray(w1a.astype(BF_NP))
    w1d = np.ascontiguousarray((SC * W1[3 * E : 3 * E + PW]).astype(BF_NP))
    w2r = np.ascontiguousarray((W2[:, 0] / SC).reshape(NT, 128).T.astype(BF_NP))
    b1r = np.ascontiguousarray((SC * b1).reshape(1, HID).astype(BF_NP))
    S = np.tile(np.eye(BS, dtype=np.float32), (1, A))

    in_maps = []
    for c in range(NCORES):
        sl = slice(c * BS, (c + 1) * BS)
        # mts[p, e, m] = bf16(men[c*BS + m, 128*e + p])
        mts = men[sl].T.reshape(KE, 128, BS).transpose(1, 0, 2).reshape(128, KE * BS)
        mts = np.ascontiguousarray(mts.astype(BF_NP))
        # amt[p, k, r] = bf16(am[c*YS + r, 128*k + p]) — this core's
        # Y_b precompute slice, k-tiled transposed
        ysl = slice(c * YS, (c + 1) * YS)
        amt = am[ysl].T.reshape(KE, 128, YS).transpose(1, 0, 2).reshape(128, KE * YS)
        amt = np.ascontiguousarray(amt.astype(BF_NP))
        pwT = pw[sl].transpose(2, 1, 0).reshape(PW, R)
        pwS = np.ascontiguousarray(np.concatenate([pwT, S], axis=0).astype(BF_NP))
        idx_r = np.ascontiguousarray(idx[sl].T.reshape(R // 128, 128).T)
        # b2 is a constant scalar on every score: fold it in here (exact)
        rough_r = np.ascontiguousarray(rough[sl].T.reshape(1, R) + np.float32(b2[0]))
        in_maps.append(
            dict(
                am=am_bf,
                amt=amt,
                idb=np.eye(128, dtype=BF_NP),
                mts=mts,
                pwS=pwS,
                idx=idx_r,
                rough=rough_r,
                w1a=w1a,
                w1b8=w1b8,
                w1c8=w1c8,
                w1d=w1d,
                w2r=w2r,
                b1r=b1r,
            )
        )
    return in_maps


def assemble_output(results):
    scores = np.empty((BATCH, A), np.float32)
    for c in range(NCORES):
        score_r = np.asarray(results[c]["out"]).reshape(A, BS)
        scores[c * BS : (c + 1) * BS, :] = score_r.T
    out = np.empty((BATCH, A + 1), np.float32)
    out[:, 0] = EPSILON
    out[:, 1:] = scores
    return out


def kernel(**inputs):
    nc = _get_nc()
    in_maps = make_in_maps(**inputs)
    res = run_bass_kernel_spmd(nc, in_maps, core_ids=list(range(NCORES)))
    return assemble_output(res.results)


if __name__ == "__main__":
    nc = build_nc()
    print("built ok")

